# revision 1
# baseline (speedup 1.0000x reference)
"""Multi-head attention (B=2,T=2048,C=1024,H=16,RoPE,causal) on 8 TRN2 cores.

Sharding: core c -> (batch b = c//4, head-group g = c%4, heads [4g,4g+4)).
Each core computes QKV projection for its 4 heads against x[b], RoPE,
causal attention in transposed-score layout [s, t], and the output
projection rows t' in [512g, 512g+512) of y[b] (the reference's
(B,H,T,Dh)->(B,T,C) reshape makes output blocks head-disjoint).
"""
import math
import sys

sys.path.insert(0, '/opt/trn_rl_repo')
sys.path.insert(0, '/opt/pypackages')

import ml_dtypes
import numpy as np
from contextlib import ExitStack

import concourse.bass as bass  # noqa: F401
import concourse.tile as tile
from concourse import bacc, mybir
from concourse.bass_utils import run_bass_kernel_spmd

BF16 = mybir.dt.bfloat16
F32 = mybir.dt.float32
NPBF16 = ml_dtypes.bfloat16

B, T, C, H, Dh = 2, 2048, 1024, 16, 64
HALF = Dh // 2          # 32
NCORES = 8
HPC = 4                 # heads per core
CPC = HPC * Dh          # channels per core = 256
SCALE = 1.0 / math.sqrt(Dh)
TT = 512                # t-tile width
NTT = T // TT           # 4
SC = 128                # s-chunk width

_compiled_nc = None


def _build_nc(dbg=False):
    nc = bacc.Bacc("TRN2", target_bir_lowering=False, debug=False)

    xT = nc.dram_tensor("xT", [C, T], BF16, kind="ExternalInput").ap()
    wqkvT = nc.dram_tensor("wqkvT", [C, 3 * CPC], BF16, kind="ExternalInput").ap()
    wpT = nc.dram_tensor("wpT", [C, C], BF16, kind="ExternalInput").ap()
    cosx = nc.dram_tensor("cosx", [128, T], BF16, kind="ExternalInput").ap()
    sinx = nc.dram_tensor("sinx", [128, T], BF16, kind="ExternalInput").ap()
    rt = nc.dram_tensor("rt", [128, 128], BF16, kind="ExternalInput").ap()
    ident = nc.dram_tensor("ident", [128, 128], F32, kind="ExternalInput").ap()
    mask01 = nc.dram_tensor("mask01", [128, 128], BF16, kind="ExternalInput").ap()
    yblk = nc.dram_tensor("yblk", [512, C], F32, kind="ExternalOutput").ap()
    if dbg:
        qdbg = nc.dram_tensor("qdbg", [128, T], BF16, kind="ExternalOutput").ap()
        kdbg = nc.dram_tensor("kdbg", [128, T], BF16, kind="ExternalOutput").ap()
        vdbg = nc.dram_tensor("vdbg", [128, T // SC, Dh + 1], BF16,
                              kind="ExternalOutput").ap()
        adbg = nc.dram_tensor("adbg", [128, 2, T], BF16,
                              kind="ExternalOutput").ap()

    with tile.TileContext(nc) as tc, ExitStack() as ctx:
        const = ctx.enter_context(tc.tile_pool(name="const", bufs=1))
        qkpool = ctx.enter_context(tc.tile_pool(name="qk", bufs=2))
        vpool = ctx.enter_context(tc.tile_pool(name="vnat", bufs=4))
        attp = ctx.enter_context(tc.tile_pool(name="attp", bufs=1))
        tmp = ctx.enter_context(tc.tile_pool(name="tmp", bufs=3))
        ahpool = ctx.enter_context(tc.tile_pool(name="ahp", bufs=9))
        psA = ctx.enter_context(tc.tile_pool(name="psA", bufs=2, space="PSUM"))
        psQK = ctx.enter_context(tc.tile_pool(name="psQK", bufs=2, space="PSUM"))
        psAT = ctx.enter_context(tc.tile_pool(name="psAT", bufs=2, space="PSUM"))

        # ---- constants (small weights first so compute can start early) ----
        wqkv_sb = const.tile([128, 8, 3 * CPC], BF16)
        nc.sync.dma_start(wqkv_sb[:], wqkvT.rearrange("(cc p) f -> p cc f", p=128))
        rt_sb = const.tile([128, 128], BF16)
        nc.sync.dma_start(rt_sb[:], rt[:])
        cos_sb = const.tile([128, T], BF16)
        nc.sync.dma_start(cos_sb[:], cosx[:])
        sin_sb = const.tile([128, T], BF16)
        nc.sync.dma_start(sin_sb[:], sinx[:])
        id_sb = const.tile([128, 128], F32)
        nc.sync.dma_start(id_sb[:], ident[:])
        mask_sb = const.tile([128, 128], BF16)
        nc.sync.dma_start(mask_sb[:], mask01[:])
        x_sb = []
        for tt in range(NTT):
            xs = const.tile([128, 8, TT], BF16, name=f"x_sb{tt}")
            nc.sync.dma_start(
                xs[:], xT.rearrange("(cc p) t -> p cc t", p=128)
                [:, :, tt * TT:(tt + 1) * TT])
            x_sb.append(xs)
        wt_sb = const.tile([128, 8, C], BF16)
        nc.sync.dma_start(wt_sb[:], wpT.rearrange("(cc p) o -> p cc o", p=128))

        att_sb = attp.tile([128, 2, T], BF16)   # [hl*64+dh, hp, t]

        for hp in range(2):
            q_sb = qkpool.tile([128, T], BF16, tag="q")
            k_sb = qkpool.tile([128, T], BF16, tag="k")
            v_nat = [vpool.tile([128, T // SC, Dh + 1], BF16, tag="vnat",
                                name=f"vnat_{hp}_{hl}")
                     for hl in range(2)]
            for hl in range(2):
                nc.gpsimd.memset(v_nat[hl][:, :, Dh:Dh + 1], 1.0)

            # ---- stage A: QKV projection + RoPE + v transpose ----
            for tt in range(NTT):
                ts = slice(tt * TT, (tt + 1) * TT)
                for gi, grp in enumerate(("q", "k", "v")):
                    f0 = gi * CPC + hp * 128
                    gps = psA.tile([128, TT], F32, tag="mm",
                                   name=f"gps_{hp}_{tt}_{gi}")
                    for cc in range(8):
                        nc.tensor.matmul(
                            gps[:], wqkv_sb[:, cc, f0:f0 + 128],
                            x_sb[tt][:, cc, :],
                            start=(cc == 0), stop=(cc == 7))
                    if grp in ("q", "k"):
                        gb = tmp.tile([128, TT], BF16, tag="gb")
                        nc.any.tensor_copy(gb[:], gps[:])
                        rot_ps = psA.tile([128, TT], F32, tag="mm",
                                          name=f"rot_{hp}_{tt}_{gi}")
                        nc.tensor.matmul(rot_ps[:], rt_sb[:], gb[:],
                                         start=True, stop=True)
                        gc = tmp.tile([128, TT], F32, tag="gc")
                        nc.gpsimd.tensor_mul(gc[:], gb[:], cos_sb[:, ts])
                        gs = tmp.tile([128, TT], F32, tag="gs")
                        nc.vector.tensor_mul(gs[:], rot_ps[:], sin_sb[:, ts])
                        dest = q_sb if grp == "q" else k_sb
                        nc.vector.tensor_add(dest[:, ts], gc[:], gs[:])
                    else:
                        vf = tmp.tile([128, TT], F32, tag="vf")
                        nc.any.tensor_copy(vf[:], gps[:])
                        for st in range(TT // 128):
                            ci = tt * 4 + st
                            tps = psA.tile([128, 128], F32, tag="mm",
                                           name=f"tps_{hp}_{tt}_{st}")
                            nc.tensor.transpose(
                                tps[:], vf[:, st * 128:(st + 1) * 128], id_sb[:])
                            nc.any.tensor_copy(v_nat[0][:, ci, 0:Dh],
                                               tps[:, 0:64])
                            nc.any.tensor_copy(v_nat[1][:, ci, 0:Dh],
                                               tps[:, 64:128])

            # ---- stage B: causal attention (scoresT layout [s, t]) ----
            for tt in range(NTT):
                ts = slice(tt * TT, (tt + 1) * TT)
                outs = [psAT.tile([Dh + 1, TT], F32, tag="attps",
                                  name=f"attps_{hp}_{tt}_{hl}")
                        for hl in range(2)]
                njs = 4 * tt + 4
                for j in range(njs):
                    sj = slice(j * SC, (j + 1) * SC)
                    qk = psQK.tile([128, 2 * TT], F32, tag="qk",
                                   name=f"qk_{hp}_{tt}_{j}")
                    for hl in range(2):
                        hb = hl * 64
                        nc.tensor.matmul(
                            qk[:, hl * TT:(hl + 1) * TT],
                            k_sb[hb:hb + 64, sj], q_sb[hb:hb + 64, ts],
                            start=True, stop=True)
                    pb = tmp.tile([128, 2 * TT], BF16, tag="probs")
                    off = (j - 4 * tt) * 128 if j >= 4 * tt else -1
                    if off <= 0:
                        # one paired exp over both heads
                        nc.scalar.activation(
                            pb[:], qk[:], mybir.ActivationFunctionType.Exp,
                            scale=SCALE)
                    else:
                        for hl in range(2):
                            h0 = hl * TT
                            nc.gpsimd.memset(pb[:, h0:h0 + off], 0.0)
                            nc.scalar.activation(
                                pb[:, h0 + off:h0 + TT], qk[:, h0 + off:h0 + TT],
                                mybir.ActivationFunctionType.Exp, scale=SCALE)
                    if off >= 0:
                        for hl in range(2):
                            dsl = slice(hl * TT + off, hl * TT + off + 128)
                            nc.vector.tensor_mul(pb[:, dsl], pb[:, dsl],
                                                 mask_sb[:])
                    for hl in range(2):
                        nc.tensor.matmul(
                            outs[hl][:], v_nat[hl][:, j, :],
                            pb[:, hl * TT:(hl + 1) * TT],
                            start=(j == 0), stop=(j == njs - 1))
                # normalize and store to att_sb
                for hl in range(2):
                    op = outs[hl]
                    zrow = tmp.tile([1, TT], F32, tag="zrow")
                    nc.vector.tensor_copy(zrow[:], op[Dh:Dh + 1, :])
                    zi = tmp.tile([1, TT], F32, tag="zi")
                    nc.vector.reciprocal_approx_fast(out=zi[:], in_=zrow[:])
                    zb = tmp.tile([64, TT], F32, tag="zb")
                    nc.gpsimd.partition_broadcast(zb[:], zi[:], channels=64)
                    nc.vector.tensor_mul(
                        att_sb[hl * 64:hl * 64 + 64, hp, ts], op[0:Dh, :], zb[:])

            if dbg and hp == 0:
                nc.sync.dma_start(qdbg[:], q_sb[:])
                nc.sync.dma_start(kdbg[:], k_sb[:])
                nc.sync.dma_start(vdbg[:], v_nat[0][:])

            # ---- output projection for this hp's two heads ----
            # Reference reshapes (B,H,T,Dh) row-major into (B,T,C): output
            # row t' = h*128 + k draws from head h positions t = 16k+j,
            # channel c' = 64j + d.  Y_h[k, o] = sum_c' A_hT[c', k] WT[c', o],
            # A_hT[64j+d, k] = attT_h[d, 16k+j].
            for hl in range(2):
                att_v = att_sb[hl * 64:hl * 64 + 64, hp, :].rearrange(
                    "d (k j) -> d k j", j=16)
                ahts = []
                for cc in range(8):
                    aht = ahpool.tile([128, 128], BF16, tag="aht",
                                      name=f"aht_{hp}_{hl}_{cc}")
                    nc.gpsimd.tensor_copy(aht[0:64, :], att_v[:, :, 2 * cc])
                    nc.gpsimd.tensor_copy(aht[64:128, :], att_v[:, :, 2 * cc + 1])
                    ahts.append(aht)
                r0 = (hp * 2 + hl) * 128
                ypss = [psA.tile([128, 512], F32, tag="mm",
                                 name=f"yps_{hp}_{hl}_{ot}") for ot in range(2)]
                for cc in range(8):
                    for ot in range(2):
                        nc.tensor.matmul(
                            ypss[ot][:], ahts[cc][:],
                            wt_sb[:, cc, ot * 512:(ot + 1) * 512],
                            start=(cc == 0), stop=(cc == 7))
                for ot in range(2):
                    yo = tmp.tile([128, 512], F32, tag="yo")
                    nc.any.tensor_copy(yo[:], ypss[ot][:])
                    nc.sync.dma_start(
                        yblk[r0:r0 + 128, ot * 512:(ot + 1) * 512], yo[:])

        if dbg:
            nc.sync.dma_start(adbg[:], att_sb[:])

    nc.compile()
    return nc


def _get_nc():
    global _compiled_nc
    if _compiled_nc is None:
        _compiled_nc = _build_nc()
    return _compiled_nc


def _host_tables():
    pos = np.arange(T, dtype=np.float32)[:, None]
    inv = np.exp(np.arange(0, Dh, 2, dtype=np.float32)
                 * (-math.log(10000.0) / Dh))
    ang = pos * inv                       # (T, 32)
    sin, cos = np.sin(ang), np.cos(ang)   # (T, 32)
    idx = np.arange(128) % HALF           # d % 32
    cos_ext = cos[:, idx].T.astype(NPBF16)  # (128, T)
    sin_ext = sin[:, idx].T.astype(NPBF16)

    R = np.zeros((128, 128), dtype=np.float32)
    for blk in (0, 64):
        for m in range(HALF):
            R[blk + m, blk + m + HALF] = -1.0
            R[blk + m + HALF, blk + m] = 1.0
    rt = np.ascontiguousarray(R.T).astype(NPBF16)

    s_i = np.arange(128)[:, None]
    t_i = np.arange(128)[None, :]
    mask01 = (t_i >= s_i).astype(np.float32).astype(NPBF16)
    ident = np.eye(128, dtype=np.float32)
    return cos_ext, sin_ext, rt, mask01, ident


def kernel(x, w_qkv, w_proj):
    x = np.asarray(x)
    w_qkv = np.asarray(w_qkv)
    w_proj = np.asarray(w_proj)
    nc = _get_nc()
    in_maps = build_in_maps(x, w_qkv, w_proj)
    res = run_bass_kernel_spmd(nc, in_maps, core_ids=list(range(NCORES)))
    y = np.zeros((B, T, C), dtype=np.float32)
    for c in range(NCORES):
        b, g = c // 4, c % 4
        y[b, 512 * g:512 * g + 512, :] = res.results[c]["yblk"]
    return y


def build_in_maps(x, w_qkv, w_proj):
    cos_ext, sin_ext, rt, mask01, ident = _host_tables()
    wq4 = w_qkv.reshape(3, H, Dh, C)
    wpT = np.ascontiguousarray(w_proj.T.astype(NPBF16))
    in_maps = []
    for c in range(NCORES):
        b, g = c // 4, c % 4
        hs = slice(4 * g, 4 * g + 4)
        wq = wq4[0, hs].reshape(CPC, C)
        wk = wq4[1, hs].reshape(CPC, C)
        wv = wq4[2, hs].reshape(CPC, C)
        wqkvT = np.concatenate([wq, wk, wv], axis=0).T.astype(NPBF16)
        xT = x[b].T.astype(NPBF16)
        in_maps.append({
            "xT": np.ascontiguousarray(xT),
            "wqkvT": np.ascontiguousarray(wqkvT),
            "wpT": wpT,
            "cosx": cos_ext, "sinx": sin_ext,
            "rt": rt, "ident": ident, "mask01": mask01,
        })
    return in_maps



# revision 8
# speedup vs baseline: 1.1731x; 1.1731x over previous
"""Multi-head attention (B=2,T=2048,C=1024,H=16,RoPE,causal) on 8 TRN2 cores.

Sharding: core c -> (batch b = c//4, head-group g = c%4, heads [4g,4g+4)).
Each core computes QKV projection for its 4 heads against x[b], RoPE,
causal attention, and the output projection rows t' in [512g, 512g+512)
of y[b] (the reference's (B,H,T,Dh)->(B,T,C) reshape makes output blocks
head-disjoint).

Schedule: software-pipelined over u = (hp, tt) slots.  Slot s emits
finalize(u=s-2), then weaves QKV/RoPE work A(s) between attention chunks
B(s-1) so the PE never drains (p-state) and exp latency is hidden.
AV uses the transposed formulation: stationary = prob chunk [s=128,t=128],
moving = V [s=128, Dh+1] (ones column gives the softmax denominator), so
each AV matmul streams only 65 rows instead of 512.  Normalization is a
per-partition tensor_scalar in the natural [t, d] layout (no partition
broadcasts), then a PE transpose restores the [d, t] layout the output
projection needs.  The Scalar engine runs exps exclusively.
"""
import math
import sys

sys.path.insert(0, '/opt/trn_rl_repo')
sys.path.insert(0, '/opt/pypackages')

import ml_dtypes
import numpy as np
from contextlib import ExitStack

import concourse.bass as bass  # noqa: F401
import concourse.tile as tile
from concourse import bacc, mybir
from concourse.bass_utils import run_bass_kernel_spmd

BF16 = mybir.dt.bfloat16
F32 = mybir.dt.float32
NPBF16 = ml_dtypes.bfloat16

B, T, C, H, Dh = 2, 2048, 1024, 16, 64
HALF = Dh // 2          # 32
NCORES = 8
HPC = 4                 # heads per core
CPC = HPC * Dh          # channels per core = 256
SCALE = 1.0 / math.sqrt(Dh)
TT = 512                # t-tile width
NTT = T // TT           # 4
SC = 128                # s-chunk width
NU = 2 * NTT            # pipeline slots: (hp, tt)

_compiled_nc = None


def _build_nc(dbg=False):
    nc = bacc.Bacc("TRN2", target_bir_lowering=False, debug=False)

    xT = nc.dram_tensor("xT", [C, T], BF16, kind="ExternalInput").ap()
    wqkvT = nc.dram_tensor("wqkvT", [C, 3 * CPC], BF16, kind="ExternalInput").ap()
    wpT = nc.dram_tensor("wpT", [C, C], BF16, kind="ExternalInput").ap()
    cosx = nc.dram_tensor("cosx", [128, T], BF16, kind="ExternalInput").ap()
    sinx = nc.dram_tensor("sinx", [128, T], BF16, kind="ExternalInput").ap()
    rt = nc.dram_tensor("rt", [128, 128], BF16, kind="ExternalInput").ap()
    ident = nc.dram_tensor("ident", [128, 128], BF16, kind="ExternalInput").ap()
    mask01 = nc.dram_tensor("mask01", [128, 128], BF16, kind="ExternalInput").ap()
    yblk = nc.dram_tensor("yblk", [512, C], F32, kind="ExternalOutput").ap()
    if dbg:
        qdbg = nc.dram_tensor("qdbg", [128, T], BF16, kind="ExternalOutput").ap()
        kdbg = nc.dram_tensor("kdbg", [128, T], BF16, kind="ExternalOutput").ap()
        vdbg = nc.dram_tensor("vdbg", [128, T // SC, Dh + 1], BF16,
                              kind="ExternalOutput").ap()
        adbg = nc.dram_tensor("adbg", [128, T], BF16,
                              kind="ExternalOutput").ap()

    with tile.TileContext(nc) as tc, ExitStack() as ctx:
        const = ctx.enter_context(tc.tile_pool(name="const", bufs=1))
        qkpool = ctx.enter_context(tc.tile_pool(name="qk", bufs=2))
        vpool = ctx.enter_context(tc.tile_pool(name="vnat", bufs=2))
        gbpool = ctx.enter_context(tc.tile_pool(name="gbp", bufs=2))
        tmp = ctx.enter_context(tc.tile_pool(name="tmp", bufs=2))
        pbpool = ctx.enter_context(tc.tile_pool(name="pbp", bufs=3))
        anat = ctx.enter_context(tc.tile_pool(name="anat", bufs=6))
        attp = ctx.enter_context(tc.tile_pool(name="attp", bufs=2))
        ahpool = ctx.enter_context(tc.tile_pool(name="ahp", bufs=1))
        yopool = ctx.enter_context(tc.tile_pool(name="yop", bufs=4))
        zipool = ctx.enter_context(tc.tile_pool(name="zip", bufs=6))
        psA = ctx.enter_context(tc.tile_pool(name="psA", bufs=2, space="PSUM"))
        psQK = ctx.enter_context(tc.tile_pool(name="psQK", bufs=2, space="PSUM"))
        psB = ctx.enter_context(tc.tile_pool(name="psB", bufs=1, space="PSUM"))

        # ---- constants (in need-order so early compute is unblocked) ----
        wqkv_sb = const.tile([128, 8, 3 * CPC], BF16)
        nc.sync.dma_start(wqkv_sb[:], wqkvT.rearrange("(cc p) f -> p cc f", p=128))
        x_sb = []
        for tt in range(NTT):
            xs = const.tile([128, 8, TT], BF16, name=f"x_sb{tt}")
            x_sb.append(xs)
        nc.sync.dma_start(
            x_sb[0][:], xT.rearrange("(cc p) t -> p cc t", p=128)[:, :, 0:TT])
        rt_sb = const.tile([128, 128], BF16)
        nc.sync.dma_start(rt_sb[:], rt[:])
        cos_sb = const.tile([128, T], BF16)
        sin_sb = const.tile([128, T], BF16)
        nc.sync.dma_start(cos_sb[:, 0:TT], cosx[:, 0:TT])
        nc.sync.dma_start(sin_sb[:, 0:TT], sinx[:, 0:TT])
        id_sb = const.tile([128, 128], BF16)
        nc.sync.dma_start(id_sb[:], ident[:])
        mask_sb = const.tile([128, 128], BF16)
        nc.sync.dma_start(mask_sb[:], mask01[:])
        for tt in range(1, NTT):
            nc.sync.dma_start(
                x_sb[tt][:],
                xT.rearrange("(cc p) t -> p cc t", p=128)
                [:, :, tt * TT:(tt + 1) * TT])
            nc.sync.dma_start(cos_sb[:, tt * TT:(tt + 1) * TT],
                              cosx[:, tt * TT:(tt + 1) * TT])
            nc.sync.dma_start(sin_sb[:, tt * TT:(tt + 1) * TT],
                              sinx[:, tt * TT:(tt + 1) * TT])
        wt_sb = const.tile([128, 8, C], BF16)

        # per-hp persistent state
        state = {}

        def alloc_hp(hp):
            q_sb = qkpool.tile([128, T], BF16, tag="q", name=f"q_sb{hp}")
            k_sb = qkpool.tile([128, T], BF16, tag="k", name=f"k_sb{hp}")
            v_nat = [vpool.tile([128, T // SC, Dh + 1], BF16, tag=f"v{hl}",
                                name=f"vnat_{hp}_{hl}")
                     for hl in range(2)]
            for hl in range(2):
                nc.gpsimd.memset(v_nat[hl][:, :, Dh:Dh + 1], 1.0)
            attT = attp.tile([128, T], BF16, tag="attT", name=f"attT{hp}")
            state[hp] = dict(q=q_sb, k=k_sb, v=v_nat, attT=attT)

        # ---------------- A(u): QKV projection + RoPE + v transpose -------
        def a_thunks(u):
            hp, tt = u // 4, u % 4
            if tt == 0:
                alloc_hp(hp)
            st = state[hp]
            ts = slice(tt * TT, (tt + 1) * TT)
            thunks = []

            def mk_qk(grp, gi):
                def th():
                    f0 = gi * CPC + hp * 128
                    gps = psA.tile([128, TT], F32, tag="mm",
                                   name=f"gps_{u}_{gi}")
                    for cc in range(8):
                        nc.tensor.matmul(
                            gps[:], wqkv_sb[:, cc, f0:f0 + 128],
                            x_sb[tt][:, cc, :],
                            start=(cc == 0), stop=(cc == 7))
                    gb = gbpool.tile([128, TT], BF16, tag=grp,
                                     name=f"gb_{u}_{grp}")
                    nc.vector.tensor_copy(gb[:], gps[:])
                    return gb
                return th

            def mk_rot(grp, gb_ref):
                def th():
                    gb = gb_ref[0]
                    rot_ps = psA.tile([128, TT], F32, tag="mm",
                                      name=f"rot_{u}_{grp}")
                    nc.tensor.matmul(rot_ps[:], rt_sb[:], gb[:],
                                     start=True, stop=True)
                    gc = tmp.tile([128, TT], F32, tag="gc" + grp,
                                  name=f"gc_{u}_{grp}")
                    nc.gpsimd.tensor_mul(gc[:], gb[:], cos_sb[:, ts])
                    gs = tmp.tile([128, TT], F32, tag="gs" + grp,
                                  name=f"gs_{u}_{grp}")
                    nc.vector.tensor_mul(gs[:], rot_ps[:], sin_sb[:, ts])
                    dest = st["q"] if grp == "q" else st["k"]
                    nc.gpsimd.tensor_add(dest[:, ts], gc[:], gs[:])
                return th

            def mk_vt(tc):
                # transposed V projection: out[t, f] directly (stationary
                # x chunk, moving w_v slice) -> no copy/transpose needed
                def th():
                    ci = tt * 4 + tc
                    f0 = 2 * CPC + hp * 128
                    psV = psA.tile([128, 128], F32, tag="mm",
                                   name=f"psV_{u}_{tc}")
                    for cc in range(8):
                        nc.tensor.matmul(
                            psV[:], x_sb[tt][:, cc, tc * 128:(tc + 1) * 128],
                            wqkv_sb[:, cc, f0:f0 + 128],
                            start=(cc == 0), stop=(cc == 7))
                    nc.vector.tensor_copy(st["v"][0][:, ci, 0:Dh],
                                          psV[:, 0:64])
                    nc.vector.tensor_copy(st["v"][1][:, ci, 0:Dh],
                                          psV[:, 64:128])
                return th

            gbq_ref, gbk_ref = [None], [None]

            def a0():
                gbq_ref[0] = mk_qk("q", 0)()
            def a1():
                gbk_ref[0] = mk_qk("k", 1)()
            thunks.append(a0)
            thunks.append(a1)
            thunks.append(mk_rot("q", gbq_ref))
            thunks.append(mk_vt(0))
            thunks.append(mk_vt(1))
            thunks.append(mk_rot("k", gbk_ref))
            thunks.append(mk_vt(2))
            thunks.append(mk_vt(3))
            return thunks

        # ---------------- B(u): causal attention chunks -------------------
        psb_state = {}

        def b_thunks(u):
            hp, tt = u // 4, u % 4
            st = state[hp]
            njs = 4 * tt + 4
            psb = [psB.tile([128, 4, Dh + 1], F32, tag=f"b{hl}",
                            name=f"psb_{u}_{hl}") for hl in range(2)]
            # start_tensor_calc pending-zeroes the whole 2KB bank, which
            # would wipe sibling tc-groups: zero once, accumulate always.
            for hl in range(2):
                nc.vector.memset(psb[hl][:], 0.0)
            psb_state[u] = psb
            pb_tiles = {}

            def mk_scores(j):
                def th():
                    off = max(0, 128 * (j - 4 * tt))
                    sj = slice(j * SC, (j + 1) * SC)
                    qk = psQK.tile([128, 2, TT], F32, tag="qk",
                                   name=f"qk_{u}_{j}")
                    for hl in range(2):
                        hb = hl * 64
                        nc.tensor.matmul(
                            qk[:, hl, off:TT],
                            st["k"][hb:hb + 64, sj],
                            st["q"][hb:hb + 64, tt * TT + off:(tt + 1) * TT],
                            start=True, stop=True)
                    pb = pbpool.tile([128, 2, TT], BF16, tag="pb",
                                     name=f"pb_{u}_{j}")
                    nc.scalar.activation(
                        pb[:, :, off:], qk[:, :, off:],
                        mybir.ActivationFunctionType.Exp, scale=SCALE)
                    if j >= 4 * tt:
                        for hl in range(2):
                            dsl = pb[:, hl, off:off + 128]
                            nc.vector.tensor_mul(dsl, dsl, mask_sb[:])
                    pb_tiles[j] = pb
                return th

            def mk_av(j):
                def th():
                    off = max(0, 128 * (j - 4 * tt))
                    pb = pb_tiles[j]
                    for tc in range(off // 128, 4):
                        for hl in range(2):
                            nc.tensor.matmul(
                                psb[hl][:, tc, :],
                                pb[:, hl, tc * 128:(tc + 1) * 128],
                                st["v"][hl][:, j, :],
                                start=False, stop=(j == 4 * tt + tc),
                                skip_group_check=True)
                return th

            thunks = [mk_scores(0)]
            for j in range(1, njs):
                def mk_pair(j):
                    s_th, a_th = mk_scores(j), mk_av(j - 1)
                    def th():
                        s_th()
                        a_th()
                    return th
                thunks.append(mk_pair(j))
            thunks.append(mk_av(njs - 1))
            return thunks

        # ---------------- F(u): normalize + transpose to attT -------------
        def f_emit(u):
            hp, tt = u // 4, u % 4
            st = state[hp]
            psb = psb_state.pop(u)
            zis, nats = {}, {}
            for hl in range(2):
                for tc in range(4):
                    zi = zipool.tile([128, 1], F32, tag="zi",
                                     name=f"zi_{u}_{hl}_{tc}")
                    nc.vector.reciprocal_approx_fast(
                        out=zi[:], in_=psb[hl][:, tc, Dh:Dh + 1])
                    nat = anat.tile([128, Dh], BF16, tag="nat",
                                    name=f"nat_{u}_{hl}_{tc}")
                    nc.vector.tensor_scalar_mul(
                        nat[:], psb[hl][:, tc, 0:Dh], zi[:])
                    zis[(hl, tc)], nats[(hl, tc)] = zi, nat
            for hl in range(2):
                hb = hl * 64
                for tc in range(4):
                    tps = psA.tile([128, 128], BF16, tag="mm",
                                   name=f"ftps_{u}_{hl}_{tc}")
                    nc.tensor.transpose(tps[0:64, :], nats[(hl, tc)][:],
                                        id_sb[:])
                    t0 = tt * TT + tc * 128
                    nc.vector.tensor_copy(
                        st["attT"][hb:hb + 64, t0:t0 + 128], tps[0:64, :])

        # ---------------- aht build + output projection -------------------
        ahts = {}

        def aht_emit(hp):
            st = state[hp]
            for hl in range(2):
                hb = hl * 64
                att_v = st["attT"][hb:hb + 64, :].rearrange(
                    "d (k j) -> d k j", j=16)
                for cc in range(8):
                    aht = ahpool.tile([128, 128], BF16,
                                      tag=f"aht{hp}_{hl}_{cc}",
                                      name=f"aht_{hp}_{hl}_{cc}")
                    nc.gpsimd.tensor_copy(aht[0:64, :], att_v[:, :, 2 * cc])
                    nc.gpsimd.tensor_copy(aht[64:128, :],
                                          att_v[:, :, 2 * cc + 1])
                    ahts[(hp, hl, cc)] = aht

        def proj_thunks(hp):
            thunks = []
            for hl in range(2):
                for ot in range(2):
                    def th(hl=hl, ot=ot):
                        r0 = (hp * 2 + hl) * 128
                        yps = psA.tile([128, 512], F32, tag="mm",
                                       name=f"yps_{hp}_{hl}_{ot}")
                        for cc in range(8):
                            nc.tensor.matmul(
                                yps[:], ahts[(hp, hl, cc)][:],
                                wt_sb[:, cc, ot * 512:(ot + 1) * 512],
                                start=(cc == 0), stop=(cc == 7))
                        yo = yopool.tile([128, 512], F32, tag="yo",
                                         name=f"yo_{hp}_{hl}_{ot}")
                        nc.vector.tensor_copy(yo[:], yps[:])
                        nc.sync.dma_start(
                            yblk[r0:r0 + 128, ot * 512:(ot + 1) * 512],
                            yo[:])
                    thunks.append(th)
            return thunks

        # ---------------- weave + slot loop -------------------------------
        def weave(primary, filler):
            if not primary:
                for f in filler:
                    f()
                return
            n, m = len(primary), len(filler)
            fi = 0
            for i, p in enumerate(primary):
                p()
                want = (i + 1) * m // n
                while fi < want:
                    filler[fi]()
                    fi += 1
            while fi < m:
                filler[fi]()
                fi += 1

        for s in range(NU + 1):
            if s == 2:
                nc.sync.dma_start(
                    wt_sb[:], wpT.rearrange("(cc p) o -> p cc o", p=128))
            if s >= 2:
                f_emit(s - 2)
            if s == 6:
                aht_emit(0)
            filler = a_thunks(s) if s < NU else proj_thunks(0)
            primary = b_thunks(s - 1) if s >= 1 else []
            weave(primary, filler)
        f_emit(NU - 1)
        aht_emit(1)
        for th in proj_thunks(1):
            th()
        if dbg:
            nc.sync.dma_start(qdbg[:], state[0]["q"][:])
            nc.sync.dma_start(kdbg[:], state[0]["k"][:])
            nc.sync.dma_start(vdbg[:], state[0]["v"][0][:])
            nc.sync.dma_start(adbg[:], state[0]["attT"][:])

    nc.compile()
    return nc


def _get_nc():
    global _compiled_nc
    if _compiled_nc is None:
        _compiled_nc = _build_nc()
    return _compiled_nc


def _host_tables():
    pos = np.arange(T, dtype=np.float32)[:, None]
    inv = np.exp(np.arange(0, Dh, 2, dtype=np.float32)
                 * (-math.log(10000.0) / Dh))
    ang = pos * inv                       # (T, 32)
    sin, cos = np.sin(ang), np.cos(ang)   # (T, 32)
    idx = np.arange(128) % HALF           # d % 32
    cos_ext = cos[:, idx].T.astype(NPBF16)  # (128, T)
    sin_ext = sin[:, idx].T.astype(NPBF16)

    R = np.zeros((128, 128), dtype=np.float32)
    for blk in (0, 64):
        for m in range(HALF):
            R[blk + m, blk + m + HALF] = -1.0
            R[blk + m + HALF, blk + m] = 1.0
    rt = np.ascontiguousarray(R.T).astype(NPBF16)

    s_i = np.arange(128)[:, None]
    t_i = np.arange(128)[None, :]
    mask01 = (t_i >= s_i).astype(np.float32).astype(NPBF16)
    ident = np.eye(128, dtype=np.float32).astype(NPBF16)
    return cos_ext, sin_ext, rt, mask01, ident


def kernel(x, w_qkv, w_proj):
    x = np.asarray(x)
    w_qkv = np.asarray(w_qkv)
    w_proj = np.asarray(w_proj)
    nc = _get_nc()
    in_maps = build_in_maps(x, w_qkv, w_proj)
    res = run_bass_kernel_spmd(nc, in_maps, core_ids=list(range(NCORES)))
    y = np.zeros((B, T, C), dtype=np.float32)
    for c in range(NCORES):
        b, g = c // 4, c % 4
        y[b, 512 * g:512 * g + 512, :] = res.results[c]["yblk"]
    return y


def build_in_maps(x, w_qkv, w_proj):
    cos_ext, sin_ext, rt, mask01, ident = _host_tables()
    wq4 = w_qkv.reshape(3, H, Dh, C)
    wpT = np.ascontiguousarray(w_proj.T.astype(NPBF16))
    in_maps = []
    for c in range(NCORES):
        b, g = c // 4, c % 4
        hs = slice(4 * g, 4 * g + 4)
        wq = wq4[0, hs].reshape(CPC, C)
        wk = wq4[1, hs].reshape(CPC, C)
        wv = wq4[2, hs].reshape(CPC, C)
        wqkvT = np.concatenate([wq, wk, wv], axis=0).T.astype(NPBF16)
        xT = x[b].T.astype(NPBF16)
        in_maps.append({
            "xT": np.ascontiguousarray(xT),
            "wqkvT": np.ascontiguousarray(wqkvT),
            "wpT": wpT,
            "cosx": cos_ext, "sinx": sin_ext,
            "rt": rt, "ident": ident, "mask01": mask01,
        })
    return in_maps
